# revision 27
# baseline (speedup 1.0000x reference)
"""Trainium2 Bass kernel for nn_BatchMegaDecode (32-layer hyena/attention hybrid, single decode step).

Strategy: 8-way tensor parallel on one trn2 chip.
- proj_W / mlp_W1 column-sharded (by attention-head / hyena-channel groups, F/8);
- out_W / mlp_W2 row(input)-sharded; partial outputs summed across cores via
  direct SBUF->SBUF remote-DMA broadcasts (XOR-relative peer addressing) plus a
  local 8-slot reduce -- no ncfw collectives on the critical path;
- FIR/IIR states and KV caches sharded by head/channel; batch replicated.
All GEMVs run as out^T = W @ x^T on the PE with weights as lhsT (pre-transposed
on the host), so channels land on SBUF partitions for the per-channel hyena ops.
"""

import sys
import numpy as np
import ml_dtypes
BF = ml_dtypes.bfloat16

for _p in ("/opt/trn_rl_repo",):
    if _p not in sys.path:
        sys.path.append(_p)

import concourse.bass as bass
import concourse.bacc as bacc
import concourse.tile as tile
import concourse.mybir as mybir
from concourse import bass_utils

L, H, NH, HD, F, B, S = 32, 1024, 16, 64, 2048, 2, 2048
HPH = H // NH
EPS = 1e-6
NC = 8
f32 = mybir.dt.float32
bf16 = mybir.dt.bfloat16
AF = mybir.ActivationFunctionType
ALU = mybir.AluOpType
AX = mybir.AxisListType

BLOCK = {0: 'HCS', 4: 'HCS', 7: 'HCS', 11: 'HCS', 14: 'HCS', 18: 'HCS', 21: 'HCS', 25: 'HCS', 28: 'HCS',
         1: 'HCM', 5: 'HCM', 8: 'HCM', 12: 'HCM', 15: 'HCM', 19: 'HCM', 22: 'HCM', 26: 'HCM', 29: 'HCM',
         2: 'HCL', 6: 'HCL', 9: 'HCL', 13: 'HCL', 16: 'HCL', 20: 'HCL', 23: 'HCL', 27: 'HCL', 30: 'HCL',
         3: 'ATT', 10: 'ATT', 17: 'ATT', 24: 'ATT', 31: 'ATT'}

SMALL_W = 448  # per-layer packed small-tensor width (f32 cols)
SW_SFW = 0     # [9]  fir1 taps, (m*3 + tap)
SW_SFB = 9     # [3]  fir1 bias' (= pb*w2 + sfb) per m; ATT: raw pb per m
SW_N1 = 12     # [8]
SW_N2 = 20     # [8]
SW_OB = 28     # [16] out_b replicated over batch, (t, b)
SW_FS = 44     # [12] fir1 state (m, b, cache)
SW_S2 = 56     # stage2 block (type-specific)


def _prep_core_inputs(inputs, c, pos):
    """Build the per-core numpy input dict for core c."""
    d = {}
    heads = slice(2 * c, 2 * c + 2)
    ch = slice(128 * c, 128 * c + 128)
    fsl = slice(256 * c, 256 * c + 256)

    x = np.asarray(inputs['x'], np.float32)
    d['xT'] = np.ascontiguousarray(x[:, 0, :].T.reshape(8, 128, 2).transpose(1, 0, 2))

    att_idx = 0
    W = pos
    T = pos // 128 + 1
    for i in range(L):
        t = BLOCK[i]
        small = np.zeros((128, SMALL_W), np.float32)
        if t == 'ATT':
            pr = np.asarray(inputs['proj_W'][i], np.float32).reshape(3, NH, HD, H)[:, heads]
            pw_c = pr.reshape(384, H)
            pb = np.asarray(inputs['proj_b'][i], np.float32).reshape(3, NH, HD)[:, heads].reshape(3, 128)
            small[:, SW_SFB:SW_SFB + 3] = pb.T
        else:
            pr = np.asarray(inputs['proj_W'][i], np.float32).reshape(NH, 3, HPH, H)[heads]
            pw_c = pr.transpose(1, 0, 2, 3).reshape(384, H)
            pb = np.asarray(inputs['proj_b'][i], np.float32).reshape(NH, 3, HPH)[heads].transpose(1, 0, 2).reshape(3, 128)
            sfw = np.asarray(inputs['sf_w'][i], np.float32).reshape(NH, 3, HPH, 3)[heads].transpose(1, 0, 2, 3).reshape(3, 128, 3)
            sfb = np.asarray(inputs['sf_b'][i], np.float32).reshape(NH, 3, HPH)[heads].transpose(1, 0, 2).reshape(3, 128)
            sfb2 = pb * sfw[:, :, 2] + sfb  # fold proj bias through tap-2
            small[:, SW_SFW:SW_SFW + 9] = sfw.transpose(1, 0, 2).reshape(128, 9)
            small[:, SW_SFB:SW_SFB + 3] = sfb2.T
            fs = np.asarray(inputs['fir_state'][i], np.float32).reshape(B, NH, 3, HPH, 2)[:, heads]
            fs = fs.transpose(2, 1, 3, 0, 4).reshape(3, 128, B, 2).transpose(1, 0, 2, 3)
            small[:, SW_FS:SW_FS + 12] = fs.reshape(128, 12)

        d[f'pwT_{i}'] = np.ascontiguousarray(pw_c.T.reshape(8, 128, 384).transpose(1, 0, 2)).astype(BF)
        small[:, SW_N1:SW_N1 + 8] = np.asarray(inputs['norm1_w'][i], np.float32).reshape(8, 128).T
        small[:, SW_N2:SW_N2 + 8] = np.asarray(inputs['norm2_w'][i], np.float32).reshape(8, 128).T
        ob = np.asarray(inputs['out_b'][i], np.float32).reshape(8, 128).T  # [128, 8]
        small[:, SW_OB:SW_OB + 16] = np.repeat(ob, 2, axis=1)

        if t == 'HCS':
            h7 = np.asarray(inputs['hcs_h'][i], np.float32)[ch]           # [128, 7]
            D = np.asarray(inputs['hcs_D'][i], np.float32)[ch]            # [128]
            st = np.asarray(inputs['hcs_state'][i], np.float32)[:, ch]    # [B, 128, 6]
            small[:, SW_S2:SW_S2 + 7] = h7
            small[:, SW_S2 + 7] = D
            small[:, SW_S2 + 8:SW_S2 + 20] = st.transpose(1, 0, 2).reshape(128, 12)
        elif t == 'HCM':
            h128 = np.asarray(inputs['hcm_h'][i], np.float32)[ch]         # [128, 128]
            D = np.asarray(inputs['hcm_D'][i], np.float32)[ch]
            st = np.asarray(inputs['hcm_state'][i], np.float32)[:, ch]    # [B, 128, 127]
            wr = h128[:, ::-1]                                            # flipped taps
            small[:, SW_S2:SW_S2 + 127] = wr[:, :127]
            small[:, SW_S2 + 127] = wr[:, 127] + D                        # gated: (+D)*u
            small[:, SW_S2 + 128:SW_S2 + 128 + 254] = st.transpose(1, 0, 2).reshape(128, 254)
        elif t == 'HCL':
            poles = np.exp(np.asarray(inputs['hcl_logpoles'][i], np.float32)[ch])  # [128, 16]
            res = np.asarray(inputs['hcl_residues'][i], np.float32)[ch]
            D = np.asarray(inputs['hcl_D'][i], np.float32)[ch]
            st = np.asarray(inputs['iir_state'][i], np.float32)[:, ch]    # [B, 128, 16]
            small[:, SW_S2:SW_S2 + 16] = poles
            small[:, SW_S2 + 16:SW_S2 + 32] = res
            small[:, SW_S2 + 32] = D
            small[:, SW_S2 + 33:SW_S2 + 65] = st.transpose(1, 0, 2).reshape(128, 32)
        else:  # ATT
            kc = np.asarray(inputs['k_cache'][att_idx], np.float32)[:, :W, heads]   # [B, W, 2, 64]
            d[f'kT_{att_idx}'] = np.ascontiguousarray(
                kc.transpose(2, 3, 0, 1).reshape(128, 2 * W)).astype(BF)
            vc = np.asarray(inputs['v_cache'][att_idx], np.float32)[:, :, heads]    # [B, S, 2, 64]
            vh = np.zeros((128, T, B, 2, 64), np.float32)
            nfull = pos // 128
            vfull = vc[:, :nfull * 128].reshape(B, nfull, 128, 2, 64).transpose(2, 1, 0, 3, 4)
            vh[:, :nfull] = vfull
            rem = pos % 128
            if rem:
                vh[:rem, nfull] = vc[:, nfull * 128:pos].transpose(1, 0, 2, 3)
            d[f'v_{att_idx}'] = np.ascontiguousarray(vh.reshape(128, T * 256)).astype(BF)
            att_idx += 1

        d[f'small_{i}'] = small
        wo = np.asarray(inputs['out_W'][i], np.float32)[:, ch]            # [1024, 128]
        d[f'owT_{i}'] = np.ascontiguousarray(wo.T).astype(BF)             # [128, 1024]
        w1 = np.asarray(inputs['mlp_W1'][i], np.float32)[fsl]             # [256, 1024]
        d[f'm1T_{i}'] = np.ascontiguousarray(w1.reshape(256, 8, 128).transpose(2, 1, 0)).astype(BF)
        w2 = np.asarray(inputs['mlp_W2'][i], np.float32)[:, fsl]          # [1024, 256]
        d[f'm2T_{i}'] = np.ascontiguousarray(w2.T.reshape(2, 128, 1024).transpose(1, 0, 2)).astype(BF)

    # constants
    cos_t = np.asarray(inputs['rope_cos'], np.float32)[pos]  # [32]
    sin_t = np.asarray(inputs['rope_sin'], np.float32)[pos]
    c64 = np.concatenate([cos_t, cos_t])
    s64 = np.concatenate([sin_t, sin_t])
    ssign = np.where(np.arange(64) < 32, -s64, s64)
    scale = HD ** -0.5
    ropec = np.stack([np.tile(c64, 2) * scale, np.tile(ssign, 2) * scale,
                      np.tile(c64, 2), np.tile(ssign, 2)], axis=1)  # [128, 4]: cq sq ck sk
    d['ropec'] = np.ascontiguousarray(ropec.astype(np.float32))
    sel2 = np.zeros((2, 128), np.float32)
    for h in range(2):
        sel2[h, h * 64:(h + 1) * 64] = 1.0
    d['sel2'] = sel2
    d['ones128'] = np.ones((128, 1), np.float32)
    d['ones1'] = np.ones((1, 128), np.float32)
    d['ident4'] = np.eye(4, dtype=np.float32).astype(BF)
    d['ident128'] = np.eye(128, dtype=np.float32).astype(BF)
    return d


STUB_ATT = False
N_LAYERS = L
USE_RDMA = False  # remote-DMA junctions (fast path) vs ncfw AllGather (fallback)


def _build(pos):
    W = pos            # cached context width
    T = pos // 128 + 1  # total s-tiles incl. current token
    rem = pos % 128     # row of the current token in tile T-1

    nc = bacc.Bacc("TRN2", target_bir_lowering=False, debug=False, num_devices=NC)

    din = {}
    def dram_in(name, shape, dt=f32):
        din[name] = nc.dram_tensor(name, list(shape), dt, kind="ExternalInput")
        return din[name]

    dram_in('xT', [128, 8, 2])
    att_idx = 0
    for i in range(L):
        dram_in(f'pwT_{i}', [128, 8, 384], bf16)
        dram_in(f'small_{i}', [128, SMALL_W])
        dram_in(f'owT_{i}', [128, 1024], bf16)
        dram_in(f'm1T_{i}', [128, 8, 256], bf16)
        dram_in(f'm2T_{i}', [128, 2, 1024], bf16)
        if BLOCK[i] == 'ATT':
            dram_in(f'kT_{att_idx}', [128, 2 * W], bf16)
            dram_in(f'v_{att_idx}', [128, T * 256], bf16)
            att_idx += 1
    for nme, shp, dt_ in [('ropec', [128, 4], f32), ('sel2', [2, 128], f32),
                          ('ones128', [128, 1], f32), ('ones1', [1, 128], f32),
                          ('ident4', [4, 4], bf16), ('ident128', [128, 128], bf16)]:
        dram_in(nme, shp, dt_)
    out_t = nc.dram_tensor('out', [2, 1024], f32, kind="ExternalOutput")

    rsem = nc.alloc_semaphore("jrsem")   # remote-arrival sem (peers inc by 2/send)
    lsem = nc.alloc_semaphore("jlsem")   # local send-complete sem

    if USE_RDMA:
        # all peers must be inside the kernel before any remote SBUF write;
        # emitted pre-Tile so the lowering's virtual clock never sees the wait
        nc.gpsimd.bir_kernel_barrier_wait([list(range(NC))])

    with tile.TileContext(nc) as tc:
        with tc.tile_pool(name="wts", bufs=3) as wp, \
             tc.tile_pool(name="wk", bufs=2) as wk, \
             tc.tile_pool(name="att", bufs=2) as ap_, \
             tc.tile_pool(name="cst", bufs=1) as cp, \
             tc.tile_pool(name="ps", bufs=1, space="PSUM") as pp, \
             tc.tile_pool(name="dram", bufs=3, space="DRAM") as dp:

            # persistent consts
            ropec = cp.tile([128, 4], f32, tag="ropec")
            sel2 = cp.tile([2, 128], f32, tag="sel2")
            ones128 = cp.tile([128, 1], f32, tag="ones128")
            ones1 = cp.tile([1, 128], f32, tag="ones1")
            ident4 = cp.tile([4, 4], bf16, tag="ident4")
            ident128 = cp.tile([128, 128], bf16, tag="ident128")
            for t_, n_ in [(ropec, 'ropec'), (sel2, 'sel2'), (ones128, 'ones128'),
                           (ones1, 'ones1'), (ident4, 'ident4'), (ident128, 'ident128')]:
                nc.sync.dma_start(out=t_[:], in_=din[n_][:, :])

            # junction landing buffers: slot 0 = own partial, slots 1..7 = peers
            # (XOR-relative: slot k holds the partial of a distinct peer).
            # Double buffered across junctions; the natural dataflow makes two
            # buffers race-free.
            jbufs = [cp.tile([128, 8, 16], f32, name=f"jbuf{v}", tag=f"jbuf{v}")
                     for v in range(2)]

            # zero-once buffers for attention
            qbd = cp.tile([128, 2, 2], bf16, tag="qbd")        # block-diag q per batch
            awT = cp.tile([128, T, 4], bf16, tag="awT")        # transposed exp-scores
            eps_t = cp.tile([1, 1], f32, tag="eps")
            nc.vector.memset(qbd[:], 0.0)
            nc.vector.memset(awT[:], 0.0)
            nc.vector.memset(eps_t[:], EPS)

            x = wk.tile([128, 8, 2], f32, tag="x")
            nc.sync.dma_start(out=x[:], in_=din['xT'][:, :, :])

            def rmsnorm_pre(x_t, small_t, w_off):
                """Deferred-rsq rmsnorm: returns (xw bf16 = x*n_w, rsq f32
                [128,2]). The GEMV may start on xw immediately; multiply its
                (tiny) output by rsq afterwards -- the sqrt chain overlaps."""
                xsq = wk.tile([128, 8, 2], f32, tag="xsq")
                nc.vector.tensor_mul(xsq[:], x_t[:], x_t[:])
                pss = pp.tile([1, 8, 2], f32, tag="misc")
                nc.tensor.matmul(pss[:], ones128[:], xsq[:].rearrange("p t b -> p (t b)"),
                                 start=True, stop=True)
                ss2 = wk.tile([1, 2], f32, tag="ss2")
                nc.vector.tensor_reduce(ss2[:], pss[:].rearrange("p t b -> p b t"),
                                        axis=AX.X, op=ALU.add)
                nc.scalar.activation(ss2[:], ss2[:], AF.Sqrt, bias=eps_t[:], scale=1.0 / H)
                psb = pp.tile([128, 2], f32, name="psb", tag="misc")
                nc.tensor.matmul(psb[:], ones1[:], ss2[:], start=True, stop=True)
                rsq = wk.tile([128, 2], f32, tag="rsq")
                nc.vector.reciprocal(rsq[:], psb[:])
                xw = wk.tile([128, 8, 2], bf16, tag="xn")
                for b in range(2):
                    nc.vector.tensor_tensor(xw[:, :, b], x_t[:, :, b],
                                            small_t[:, w_off:w_off + 8], op=ALU.mult)
                return xw, rsq

            JJ = [0]
            # waits attached AFTER TileContext exits: the tile scheduling sim
            # cannot model remote semaphore increments (would flag deadlock)
            pending_waits = []

            def junction_cc(psum_p, x_t, add_ob, small_t):
                """Fallback: AllGather partials via ncfw collective + local sum."""
                stage = wk.tile([128, 16], f32, tag="jstage")
                nc.vector.tensor_copy(stage[:], psum_p[:].rearrange("p t b -> p (t b)"))
                jin = dp.tile([128, 16], f32, tag="jin")
                # SWDGE (gpsimd) DMA: the collective trigger runs on gpsimd
                # too, so the done-sem wait is a same-engine poll instead of a
                # ~2us cross-engine wakeup before PSEUDO_TRIGGER_COLLECTIVE
                nc.gpsimd.dma_start(out=jin[:], in_=stage[:])
                jout = dp.tile([1024, 16], f32, tag="jout")
                nc.gpsimd.collective_compute(
                    "AllGather", ALU.bypass,
                    replica_groups=[list(range(NC))],
                    ins=[jin.opt()], outs=[jout.opt()],
                )
                land = wk.tile([128, 8, 16], f32, tag="land")
                # gpsimd is already awake right after the collective trigger;
                # issuing the landing DMA there skips a Sync-engine wakeup
                nc.gpsimd.dma_start(out=land[:], in_=jout[:, :].rearrange("(r p) f -> p r f", p=128))
                txo = None
                if add_ob:
                    txo = wk.tile([128, 16], f32, tag="txo")
                    nc.vector.tensor_add(txo[:], x_t[:].rearrange("p t b -> p (t b)"),
                                         small_t[:, SW_OB:SW_OB + 16])
                red = wk.tile([128, 16], f32, tag="red")
                nc.vector.tensor_reduce(red[:], land[:].rearrange("p r f -> p f r"),
                                        axis=AX.X, op=ALU.add)
                nx = wk.tile([128, 8, 2], f32, tag="x")
                nc.vector.tensor_add(nx[:].rearrange("p t b -> p (t b)"), red[:],
                                     txo[:] if txo is not None else
                                     x_t[:].rearrange("p t b -> p (t b)"))
                return nx

            def junction(psum_p, x_t, add_ob, small_t):
                """Cross-core sum of partials via SBUF->SBUF remote-DMA broadcast."""
                if not USE_RDMA:
                    return junction_cc(psum_p, x_t, add_ob, small_t)
                jj = JJ[0]
                JJ[0] += 1
                buf = jbufs[jj % 2]
                # Preps read buf[:,0] (in_ap) -> Tile RAW edge on this copy
                # gates the whole Pool stream (preps, then trigger).
                nc.vector.tensor_copy(buf[:, 0, :], psum_p[:].rearrange("p t b -> p (t b)"))
                for k in range(1, NC):
                    rd = [None] * 8
                    rd[k] = (0, k)
                    nc.gpsimd.remote_dma_broadcast(
                        out_ap=buf[:, k, :], in_ap=buf[:, 0, :],
                        remote_sem=rsem, local_sem=lsem, rdests=rd)
                nc.gpsimd.trigger_dma(count=None)
                txo = None
                if add_ob:
                    txo = wk.tile([128, 16], f32, tag="txo")
                    nc.vector.tensor_add(txo[:], x_t[:].rearrange("p t b -> p (t b)"),
                                         small_t[:, SW_OB:SW_OB + 16])
                red = wk.tile([128, 16], f32, tag="red")
                rinst = nc.vector.tensor_reduce(red[:], buf[:].rearrange("p s f -> p f s"),
                                                axis=AX.X, op=ALU.add)
                pending_waits.append((rinst, rsem, 14 * (jj + 1)))
                nx = wk.tile([128, 8, 2], f32, tag="x")
                nc.vector.tensor_add(nx[:].rearrange("p t b -> p (t b)"), red[:],
                                     txo[:] if txo is not None else
                                     x_t[:].rearrange("p t b -> p (t b)"))
                return nx

            att_idx = 0
            for i in range(N_LAYERS):
                bt = BLOCK[i]
                pwT = wp.tile([128, 8, 384], bf16, tag="pwT")
                nc.sync.dma_start(out=pwT[:], in_=din[f'pwT_{i}'][:, :, :])
                small = wp.tile([128, SMALL_W], f32, tag="small")
                nc.sync.dma_start(out=small[:], in_=din[f'small_{i}'][:, :])
                owT = wp.tile([128, 1024], bf16, tag="owT")
                nc.sync.dma_start(out=owT[:], in_=din[f'owT_{i}'][:, :])
                m1T = wp.tile([128, 8, 256], bf16, tag="m1T")
                nc.sync.dma_start(out=m1T[:], in_=din[f'm1T_{i}'][:, :, :])
                m2T = wp.tile([128, 2, 1024], bf16, tag="m2T")
                nc.sync.dma_start(out=m2T[:], in_=din[f'm2T_{i}'][:, :, :])
                if bt == 'ATT':
                    kT = ap_.tile([128, 2 * W], bf16, tag="kT")
                    nc.sync.dma_start(out=kT[:], in_=din[f'kT_{att_idx}'][:, :])
                    vv = ap_.tile([128, T, 2, 2, 64], bf16, tag="vv")
                    nc.sync.dma_start(out=vv[:], in_=din[f'v_{att_idx}'][:, :].rearrange(
                        "p (t b h d) -> p t b h d", t=T, b=2, h=2))

                xn, rsq1 = rmsnorm_pre(x, small, SW_N1)

                # proj: z_raw^T [128, 3(m), 2(b)] (norm scale applied below)
                pz_raw = pp.tile([128, 3, 2], f32, tag="zh")
                for m in range(3):
                    for kt in range(8):
                        nc.tensor.matmul(pz_raw[:, m, :], pwT[:, kt, m * 128:(m + 1) * 128],
                                         xn[:, kt, :], start=(kt == 0), stop=(kt == 7))
                pz = wk.tile([128, 3, 2], f32, tag="zs")
                for m in range(3):
                    nc.vector.tensor_mul(pz[:, m, :], pz_raw[:, m, :], rsq[:] if False else rsq1[:])

                if bt != 'ATT':
                    # fir1 on each of x2|x1|v tiles: zp = w2*u + s0*w0 + s1*w1 + sfb'
                    zp = wk.tile([128, 3, 2], f32, tag="zp")
                    tt = wk.tile([128, 2], f32, tag="tt")
                    for m in range(3):
                        nc.vector.tensor_scalar(tt[:], pz[:, m, :],
                                                small[:, SW_SFW + 3 * m + 2:SW_SFW + 3 * m + 3], small[:, SW_SFB + m:SW_SFB + m + 1],
                                                op0=ALU.mult, op1=ALU.add)
                        nc.vector.scalar_tensor_tensor(
                            tt[:], small[:, SW_FS + 4 * m:SW_FS + 4 * m + 4:2],
                            small[:, SW_SFW + 3 * m:SW_SFW + 3 * m + 1], tt[:], op0=ALU.mult, op1=ALU.add)
                        nc.vector.scalar_tensor_tensor(
                            zp[:, m, :], small[:, SW_FS + 4 * m + 1:SW_FS + 4 * m + 5:2],
                            small[:, SW_SFW + 3 * m + 1:SW_SFW + 3 * m + 2], tt[:], op0=ALU.mult, op1=ALU.add)
                    x1v = wk.tile([128, 2], f32, tag="x1v")
                    nc.vector.tensor_mul(x1v[:], zp[:, 1, :], zp[:, 2, :])

                    y2 = wk.tile([128, 2], bf16, tag="y2")
                    if bt == 'HCS':
                        acc = wk.tile([128, 2], f32, tag="acc")
                        yb = wk.tile([128, 2], f32, tag="yb")
                        scratch = wk.tile([128, 6], f32, tag="scr6")
                        nc.vector.tensor_scalar(yb[:], x1v[:], small[:, SW_S2 + 6:SW_S2 + 7],
                                                small[:, SW_S2 + 7:SW_S2 + 8],
                                                op0=ALU.mult, op1=ALU.add)
                        for b in range(2):
                            nc.vector.tensor_mul(scratch[:], small[:, SW_S2 + 8 + 6 * b:SW_S2 + 14 + 6 * b],
                                                 small[:, SW_S2:SW_S2 + 6])
                            nc.vector.tensor_reduce(acc[:, b:b + 1], scratch[:], axis=AX.X, op=ALU.add)
                        nc.vector.tensor_add(yb[:], yb[:], acc[:])
                        nc.vector.tensor_mul(y2[:], yb[:], zp[:, 0, :])
                    elif bt == 'HCM':
                        acc = wk.tile([128, 2], f32, tag="acc")
                        yb = wk.tile([128, 2], f32, tag="yb")
                        scratch = wk.tile([128, 127], f32, tag="scr127")
                        for b in range(2):
                            nc.vector.tensor_mul(scratch[:], small[:, SW_S2 + 128 + 127 * b:SW_S2 + 255 + 127 * b],
                                                 small[:, SW_S2:SW_S2 + 127])
                            nc.vector.tensor_reduce(acc[:, b:b + 1], scratch[:], axis=AX.X, op=ALU.add)
                        nc.vector.scalar_tensor_tensor(yb[:], x1v[:], small[:, SW_S2 + 127:SW_S2 + 128],
                                                       acc[:], op0=ALU.mult, op1=ALU.add)
                        nc.vector.tensor_mul(y2[:], yb[:], zp[:, 0, :])
                    else:  # HCL
                        dx = wk.tile([128, 2], f32, tag="dx")
                        nc.vector.tensor_scalar_mul(dx[:], x1v[:], small[:, SW_S2 + 32:SW_S2 + 33])
                        t1 = wk.tile([128, 16], f32, tag="t1")
                        iirn = wk.tile([128, 16], f32, tag="iirn")
                        res = wk.tile([128, 2], f32, tag="res")
                        for b in range(2):
                            nc.vector.tensor_mul(t1[:], small[:, SW_S2 + 33 + 16 * b:SW_S2 + 49 + 16 * b],
                                                 small[:, SW_S2:SW_S2 + 16])
                            nc.vector.tensor_scalar_add(iirn[:], t1[:], x1v[:, b:b + 1])
                            nc.vector.tensor_mul(t1[:], iirn[:], small[:, SW_S2 + 16:SW_S2 + 32])
                            nc.vector.tensor_reduce(res[:, b:b + 1], t1[:], axis=AX.X, op=ALU.add)
                        nc.vector.tensor_add(res[:], res[:], dx[:])
                        nc.vector.tensor_mul(y2[:], res[:], zp[:, 0, :])
                elif STUB_ATT:
                    y2 = wk.tile([128, 2], bf16, tag="y2")
                    nc.vector.tensor_copy(y2[:], pz[:, 0, :])
                else:
                    # ---- attention ----
                    q_sb = wk.tile([128, 2], f32, tag="q_sb")
                    k_sb = wk.tile([128, 2], f32, tag="k_sb")
                    v_sb = wk.tile([128, 2], bf16, tag="v_sb")
                    for m, dst in ((0, q_sb), (1, k_sb), (2, v_sb)):
                        nc.vector.tensor_scalar_add(dst[:], pz[:, m, :], small[:, SW_SFB + m:SW_SFB + m + 1])

                    def rope(src, c_col, s_col, dt, nm):
                        tmp = wk.tile([128, 2], f32, tag="rtmp")
                        for base in (0, 64):
                            nc.vector.tensor_copy(tmp[base:base + 32, :], src[base + 32:base + 64, :])
                            nc.vector.tensor_copy(tmp[base + 32:base + 64, :], src[base:base + 32, :])
                        nc.vector.tensor_scalar_mul(tmp[:], tmp[:], ropec[:, s_col:s_col + 1])
                        dst = wk.tile([128, 2], dt, name=nm, tag=nm)
                        nc.vector.scalar_tensor_tensor(dst[:], src[:], ropec[:, c_col:c_col + 1], tmp[:],
                                                       op0=ALU.mult, op1=ALU.add)
                        return dst

                    qr = rope(q_sb, 0, 1, f32, "qr")
                    kr = rope(k_sb, 2, 3, bf16, "kr")

                    nc.vector.tensor_copy(qbd[0:64, 0, 0:1], qr[0:64, 0:1])
                    nc.vector.tensor_copy(qbd[64:128, 0, 1:2], qr[64:128, 0:1])
                    nc.vector.tensor_copy(qbd[0:64, 1, 0:1], qr[0:64, 1:2])
                    nc.vector.tensor_copy(qbd[64:128, 1, 1:2], qr[64:128, 1:2])

                    pscs = [pp.tile([2, 1024], f32, name=f"psc{b}", tag=f"sc{b}") for b in range(2)]
                    for b in range(2):
                        for c0 in range(0, W, 512):
                            cw = min(512, W - c0)
                            nc.tensor.matmul(pscs[b][:, c0:c0 + cw],
                                             qbd[:, b, :], kT[:, b * W + c0:b * W + c0 + cw],
                                             start=True, stop=True)
                    pcur = pp.tile([2, 2], f32, name="pcur", tag="misc")
                    for b in range(2):
                        nc.tensor.matmul(pcur[:, b:b + 1], qbd[:, b, :], kr[:, b:b + 1],
                                         start=True, stop=True)

                    escs = [wk.tile([2, W + 1], bf16, name=f"esc{b}", tag=f"esc{b}") for b in range(2)]
                    rec = wk.tile([2, 2], f32, tag="rec")
                    mx = wk.tile([2, 2], f32, tag="mx")
                    nm = wk.tile([2, 2], f32, tag="nm")
                    se = wk.tile([2, 2], f32, tag="se")
                    ecur = wk.tile([2, 2], f32, tag="ecur")
                    for b in range(2):
                        nc.vector.tensor_reduce(mx[:, b:b + 1], pscs[b][:, 0:W], axis=AX.X, op=ALU.max)
                        nc.vector.tensor_tensor(mx[:, b:b + 1], mx[:, b:b + 1], pcur[:, b:b + 1], op=ALU.max)
                        nc.vector.tensor_scalar_mul(nm[:, b:b + 1], mx[:, b:b + 1], -1.0)
                        nc.scalar.activation(escs[b][:, 0:W], pscs[b][:, 0:W], AF.Exp,
                                             bias=nm[:, b:b + 1], scale=1.0, accum_out=se[:, b:b + 1])
                        nc.scalar.activation(escs[b][:, W:W + 1], pcur[:, b:b + 1], AF.Exp,
                                             bias=nm[:, b:b + 1], scale=1.0)
                        nc.vector.tensor_copy(ecur[:, b:b + 1], escs[b][:, W:W + 1])
                        nc.vector.tensor_add(se[:, b:b + 1], se[:, b:b + 1], ecur[:, b:b + 1])
                        nc.vector.reciprocal(rec[:, b:b + 1], se[:, b:b + 1])

                    pvt = pp.tile([1, 256], bf16, name="pvt", tag="misc")
                    for b in range(2):
                        nc.tensor.matmul(pvt[:, b * 128:(b + 1) * 128],
                                         v_sb[:, b:b + 1], ident128[:],
                                         is_transpose=True, start=True, stop=True)
                    nc.vector.tensor_copy(
                        vv[rem:rem + 1, T - 1, :, :, :].rearrange("p b h d -> p (b h d)"),
                        pvt[:, :])

                    for t_ in range(T):
                        c0 = t_ * 128
                        cwid = min(128, W + 1 - c0)
                        for b in range(2):
                            pat = pp.tile([128, 2], bf16, name=f"pat{b}", tag="misc")
                            nc.tensor.matmul(pat[0:cwid, :], escs[b][:, c0:c0 + cwid],
                                             ident4[0:2, 0:2],
                                             is_transpose=True, start=True, stop=True)
                            nc.vector.tensor_copy(awT[0:cwid, t_, 2 * b:2 * b + 2], pat[0:cwid, :])

                    py = pp.tile([128, 2], f32, name="py", tag="ctxy")
                    for b in range(2):
                        for h in range(2):
                            for t_ in range(T):
                                nc.tensor.matmul(py[h * 64:(h + 1) * 64, b:b + 1],
                                                 vv[:, t_, b, h, :], awT[:, t_, b * 2 + h:b * 2 + h + 1],
                                                 start=(t_ == 0), stop=(t_ == T - 1))

                    prb = pp.tile([128, 2], f32, name="prb", tag="misc")
                    nc.tensor.matmul(prb[:], sel2[:], rec[:], start=True, stop=True)
                    rb = wk.tile([128, 2], f32, tag="rbs")
                    nc.vector.tensor_copy(rb[:], prb[:])
                    y2 = wk.tile([128, 2], bf16, tag="y2")
                    nc.vector.tensor_mul(y2[:], py[:], rb[:])

                # out proj: partial^T [128, 8(t), 2(b)]
                pp1 = pp.tile([128, 8, 2], f32, tag="pj")
                for m in range(8):
                    nc.tensor.matmul(pp1[:, m, :], owT[:, m * 128:(m + 1) * 128], y2[:],
                                     start=True, stop=True)
                x_mid = junction(pp1, x, True, small)

                # mlp
                xn2, rsq2 = rmsnorm_pre(x_mid, small, SW_N2)
                ph = pp.tile([128, 2, 2], f32, name="ph", tag="zh")
                for m in range(2):
                    for kt in range(8):
                        nc.tensor.matmul(ph[:, m, :], m1T[:, kt, m * 128:(m + 1) * 128],
                                         xn2[:, kt, :], start=(kt == 0), stop=(kt == 7))
                hs = wk.tile([128, 2, 2], f32, tag="hs")
                for m in range(2):
                    nc.vector.tensor_mul(hs[:, m, :], ph[:, m, :], rsq2[:])
                hg = wk.tile([128, 2, 2], bf16, tag="hg")
                for m in range(2):
                    nc.scalar.activation(hg[:, m, :], hs[:, m, :], AF.Gelu_apprx_tanh)
                pp2 = pp.tile([128, 8, 2], f32, name="pp2", tag="pj")
                for m in range(8):
                    for kt in range(2):
                        nc.tensor.matmul(pp2[:, m, :], m2T[:, kt, m * 128:(m + 1) * 128],
                                         hg[:, kt, :], start=(kt == 0), stop=(kt == 1))
                x = junction(pp2, x_mid, False, small)

                if bt == 'ATT':
                    att_idx += 1

            for b in range(2):
                nc.sync.dma_start(out=out_t.ap()[b].rearrange("(t p) -> p t", p=128),
                                  in_=x[:, :, b])

    # attach the remote-arrival gates now that the tile scheduler has run;
    # check=False: extra waits are split onto EventSemaphores by compile()
    for rinst, sem, thresh in pending_waits:
        rinst.wait_op(sem, thresh, "sem-ge", check=False)

    nc.compile()
    return nc


_CACHE = {}


def kernel(**inputs):
    pos = int(np.asarray(inputs['position']))
    if pos not in _CACHE:
        _CACHE[pos] = _build(pos)
    nc = _CACHE[pos]
    in_maps = [_prep_core_inputs(inputs, c, pos) for c in range(NC)]
    res = bass_utils.run_bass_kernel_spmd(nc, in_maps, core_ids=list(range(NC)))
    out = res.results[0]['out']  # [2, 1024], replicated across cores
    return out.reshape(B, 1, H).astype(np.float32)


# revision 32
# speedup vs baseline: 1.0254x; 1.0254x over previous
"""Trainium2 Bass kernel for nn_BatchMegaDecode (32-layer hyena/attention hybrid, single decode step).

Strategy: 8-way tensor parallel on one trn2 chip.
- proj_W / mlp_W1 column-sharded (by attention-head / hyena-channel groups, F/8);
- out_W / mlp_W2 row(input)-sharded; partial outputs summed across cores via an
  ncfw AllGather of partials + local 8-slot reduce (a remote-DMA SBUF->SBUF
  variant was implemented and measured at ~55us/hop steady-state -- 4x worse
  than the collective -- and is kept behind USE_RDMA=False);
- FIR/IIR states and KV caches sharded by head/channel; batch replicated.
All GEMVs run as out^T = W @ x^T on the PE with weights as lhsT (pre-transposed
on the host), so channels land on SBUF partitions for the per-channel hyena ops.
RMSNorm's rsqrt is deferred: the PE consumes x*norm_w immediately and the 1/rms
scale is applied to the small projected z, so the sqrt chain overlaps the GEMV.
The junction's staging DMA runs on gpsimd's SWDGE queue so the collective
trigger (also gpsimd) polls its completion on the same engine.
"""

import sys
import numpy as np
import ml_dtypes
BF = ml_dtypes.bfloat16

for _p in ("/opt/trn_rl_repo",):
    if _p not in sys.path:
        sys.path.append(_p)

import concourse.bass as bass
import concourse.bacc as bacc
import concourse.tile as tile
import concourse.mybir as mybir
from concourse import bass_utils

L, H, NH, HD, F, B, S = 32, 1024, 16, 64, 2048, 2, 2048
HPH = H // NH
EPS = 1e-6
NC = 8
f32 = mybir.dt.float32
bf16 = mybir.dt.bfloat16
AF = mybir.ActivationFunctionType
ALU = mybir.AluOpType
AX = mybir.AxisListType

BLOCK = {0: 'HCS', 4: 'HCS', 7: 'HCS', 11: 'HCS', 14: 'HCS', 18: 'HCS', 21: 'HCS', 25: 'HCS', 28: 'HCS',
         1: 'HCM', 5: 'HCM', 8: 'HCM', 12: 'HCM', 15: 'HCM', 19: 'HCM', 22: 'HCM', 26: 'HCM', 29: 'HCM',
         2: 'HCL', 6: 'HCL', 9: 'HCL', 13: 'HCL', 16: 'HCL', 20: 'HCL', 23: 'HCL', 27: 'HCL', 30: 'HCL',
         3: 'ATT', 10: 'ATT', 17: 'ATT', 24: 'ATT', 31: 'ATT'}

SMALL_W = 448  # per-layer packed small-tensor width (f32 cols)
SW_SFW = 0     # [9]  fir1 taps, (m*3 + tap)
SW_SFB = 9     # [3]  fir1 bias' (= pb*w2 + sfb) per m; ATT: raw pb per m
SW_N1 = 12     # [8]
SW_N2 = 20     # [8]
SW_OB = 28     # [16] out_b replicated over batch, (t, b)
SW_FS = 44     # [12] fir1 state (m, b, cache)
SW_S2 = 56     # stage2 block (type-specific)


def _prep_core_inputs(inputs, c, pos):
    """Build the per-core numpy input dict for core c."""
    d = {}
    heads = slice(2 * c, 2 * c + 2)
    ch = slice(128 * c, 128 * c + 128)
    fsl = slice(256 * c, 256 * c + 256)

    x = np.asarray(inputs['x'], np.float32)
    d['xT'] = np.ascontiguousarray(x[:, 0, :].T.reshape(8, 128, 2).transpose(1, 0, 2))

    att_idx = 0
    W = pos
    T = pos // 128 + 1
    for i in range(L):
        t = BLOCK[i]
        small = np.zeros((128, SMALL_W), np.float32)
        if t == 'ATT':
            pr = np.asarray(inputs['proj_W'][i], np.float32).reshape(3, NH, HD, H)[:, heads]
            pw_c = pr.reshape(384, H)
            pb = np.asarray(inputs['proj_b'][i], np.float32).reshape(3, NH, HD)[:, heads].reshape(3, 128)
            small[:, SW_SFB:SW_SFB + 3] = pb.T
        else:
            pr = np.asarray(inputs['proj_W'][i], np.float32).reshape(NH, 3, HPH, H)[heads]
            pw_c = pr.transpose(1, 0, 2, 3).reshape(384, H)
            pb = np.asarray(inputs['proj_b'][i], np.float32).reshape(NH, 3, HPH)[heads].transpose(1, 0, 2).reshape(3, 128)
            sfw = np.asarray(inputs['sf_w'][i], np.float32).reshape(NH, 3, HPH, 3)[heads].transpose(1, 0, 2, 3).reshape(3, 128, 3)
            sfb = np.asarray(inputs['sf_b'][i], np.float32).reshape(NH, 3, HPH)[heads].transpose(1, 0, 2).reshape(3, 128)
            sfb2 = pb * sfw[:, :, 2] + sfb  # fold proj bias through tap-2
            small[:, SW_SFW:SW_SFW + 9] = sfw.transpose(1, 0, 2).reshape(128, 9)
            small[:, SW_SFB:SW_SFB + 3] = sfb2.T
            fs = np.asarray(inputs['fir_state'][i], np.float32).reshape(B, NH, 3, HPH, 2)[:, heads]
            fs = fs.transpose(2, 1, 3, 0, 4).reshape(3, 128, B, 2).transpose(1, 0, 2, 3)
            small[:, SW_FS:SW_FS + 12] = fs.reshape(128, 12)

        d[f'pwT_{i}'] = np.ascontiguousarray(pw_c.T.reshape(8, 128, 384).transpose(1, 0, 2)).astype(BF)
        small[:, SW_N1:SW_N1 + 8] = np.asarray(inputs['norm1_w'][i], np.float32).reshape(8, 128).T
        small[:, SW_N2:SW_N2 + 8] = np.asarray(inputs['norm2_w'][i], np.float32).reshape(8, 128).T
        ob = np.asarray(inputs['out_b'][i], np.float32).reshape(8, 128).T  # [128, 8]
        small[:, SW_OB:SW_OB + 16] = np.repeat(ob, 2, axis=1)

        if t == 'HCS':
            h7 = np.asarray(inputs['hcs_h'][i], np.float32)[ch]           # [128, 7]
            D = np.asarray(inputs['hcs_D'][i], np.float32)[ch]            # [128]
            st = np.asarray(inputs['hcs_state'][i], np.float32)[:, ch]    # [B, 128, 6]
            small[:, SW_S2:SW_S2 + 7] = h7
            small[:, SW_S2 + 7] = D
            small[:, SW_S2 + 8:SW_S2 + 20] = st.transpose(1, 0, 2).reshape(128, 12)
        elif t == 'HCM':
            h128 = np.asarray(inputs['hcm_h'][i], np.float32)[ch]         # [128, 128]
            D = np.asarray(inputs['hcm_D'][i], np.float32)[ch]
            st = np.asarray(inputs['hcm_state'][i], np.float32)[:, ch]    # [B, 128, 127]
            wr = h128[:, ::-1]                                            # flipped taps
            small[:, SW_S2:SW_S2 + 127] = wr[:, :127]
            small[:, SW_S2 + 127] = wr[:, 127] + D                        # gated: (+D)*u
            small[:, SW_S2 + 128:SW_S2 + 128 + 254] = st.transpose(1, 0, 2).reshape(128, 254)
        elif t == 'HCL':
            poles = np.exp(np.asarray(inputs['hcl_logpoles'][i], np.float32)[ch])  # [128, 16]
            res = np.asarray(inputs['hcl_residues'][i], np.float32)[ch]
            D = np.asarray(inputs['hcl_D'][i], np.float32)[ch]
            st = np.asarray(inputs['iir_state'][i], np.float32)[:, ch]    # [B, 128, 16]
            small[:, SW_S2:SW_S2 + 16] = poles
            small[:, SW_S2 + 16:SW_S2 + 32] = res
            small[:, SW_S2 + 32] = D
            small[:, SW_S2 + 33:SW_S2 + 65] = st.transpose(1, 0, 2).reshape(128, 32)
        else:  # ATT
            kc = np.asarray(inputs['k_cache'][att_idx], np.float32)[:, :W, heads]   # [B, W, 2, 64]
            d[f'kT_{att_idx}'] = np.ascontiguousarray(
                kc.transpose(2, 3, 0, 1).reshape(128, 2 * W)).astype(BF)
            vc = np.asarray(inputs['v_cache'][att_idx], np.float32)[:, :, heads]    # [B, S, 2, 64]
            vh = np.zeros((128, T, B, 2, 64), np.float32)
            nfull = pos // 128
            vfull = vc[:, :nfull * 128].reshape(B, nfull, 128, 2, 64).transpose(2, 1, 0, 3, 4)
            vh[:, :nfull] = vfull
            rem = pos % 128
            if rem:
                vh[:rem, nfull] = vc[:, nfull * 128:pos].transpose(1, 0, 2, 3)
            d[f'v_{att_idx}'] = np.ascontiguousarray(vh.reshape(128, T * 256)).astype(BF)
            att_idx += 1

        d[f'small_{i}'] = small
        wo = np.asarray(inputs['out_W'][i], np.float32)[:, ch]            # [1024, 128]
        d[f'owT_{i}'] = np.ascontiguousarray(wo.T).astype(BF)             # [128, 1024]
        w1 = np.asarray(inputs['mlp_W1'][i], np.float32)[fsl]             # [256, 1024]
        d[f'm1T_{i}'] = np.ascontiguousarray(w1.reshape(256, 8, 128).transpose(2, 1, 0)).astype(BF)
        w2 = np.asarray(inputs['mlp_W2'][i], np.float32)[:, fsl]          # [1024, 256]
        d[f'm2T_{i}'] = np.ascontiguousarray(w2.T.reshape(2, 128, 1024).transpose(1, 0, 2)).astype(BF)

    # constants
    cos_t = np.asarray(inputs['rope_cos'], np.float32)[pos]  # [32]
    sin_t = np.asarray(inputs['rope_sin'], np.float32)[pos]
    c64 = np.concatenate([cos_t, cos_t])
    s64 = np.concatenate([sin_t, sin_t])
    ssign = np.where(np.arange(64) < 32, -s64, s64)
    scale = HD ** -0.5
    ropec = np.stack([np.tile(c64, 2) * scale, np.tile(ssign, 2) * scale,
                      np.tile(c64, 2), np.tile(ssign, 2)], axis=1)  # [128, 4]: cq sq ck sk
    d['ropec'] = np.ascontiguousarray(ropec.astype(np.float32))
    sel2 = np.zeros((2, 128), np.float32)
    for h in range(2):
        sel2[h, h * 64:(h + 1) * 64] = 1.0
    d['sel2'] = sel2
    d['ones128'] = np.ones((128, 1), np.float32)
    d['ones1'] = np.ones((1, 128), np.float32)
    d['ident4'] = np.eye(4, dtype=np.float32).astype(BF)
    d['ident128'] = np.eye(128, dtype=np.float32).astype(BF)
    return d


STUB_ATT = False
N_LAYERS = L
USE_RDMA = False  # remote-DMA junctions (fast path) vs ncfw AllGather (fallback)


def _build(pos):
    W = pos            # cached context width
    T = pos // 128 + 1  # total s-tiles incl. current token
    rem = pos % 128     # row of the current token in tile T-1

    nc = bacc.Bacc("TRN2", target_bir_lowering=False, debug=False, num_devices=NC)

    din = {}
    def dram_in(name, shape, dt=f32):
        din[name] = nc.dram_tensor(name, list(shape), dt, kind="ExternalInput")
        return din[name]

    dram_in('xT', [128, 8, 2])
    att_idx = 0
    for i in range(L):
        dram_in(f'pwT_{i}', [128, 8, 384], bf16)
        dram_in(f'small_{i}', [128, SMALL_W])
        dram_in(f'owT_{i}', [128, 1024], bf16)
        dram_in(f'm1T_{i}', [128, 8, 256], bf16)
        dram_in(f'm2T_{i}', [128, 2, 1024], bf16)
        if BLOCK[i] == 'ATT':
            dram_in(f'kT_{att_idx}', [128, 2 * W], bf16)
            dram_in(f'v_{att_idx}', [128, T * 256], bf16)
            att_idx += 1
    for nme, shp, dt_ in [('ropec', [128, 4], f32), ('sel2', [2, 128], f32),
                          ('ones128', [128, 1], f32), ('ones1', [1, 128], f32),
                          ('ident4', [4, 4], bf16), ('ident128', [128, 128], bf16)]:
        dram_in(nme, shp, dt_)
    out_t = nc.dram_tensor('out', [2, 1024], f32, kind="ExternalOutput")

    rsem = nc.alloc_semaphore("jrsem")   # remote-arrival sem (peers inc by 2/send)
    lsem = nc.alloc_semaphore("jlsem")   # local send-complete sem

    if USE_RDMA:
        # all peers must be inside the kernel before any remote SBUF write;
        # emitted pre-Tile so the lowering's virtual clock never sees the wait
        nc.gpsimd.bir_kernel_barrier_wait([list(range(NC))])

    with tile.TileContext(nc) as tc:
        with tc.tile_pool(name="wts", bufs=4) as wp, \
             tc.tile_pool(name="wk", bufs=2) as wk, \
             tc.tile_pool(name="att", bufs=2) as ap_, \
             tc.tile_pool(name="cst", bufs=1) as cp, \
             tc.tile_pool(name="ps", bufs=1, space="PSUM") as pp, \
             tc.tile_pool(name="dram", bufs=3, space="DRAM") as dp:

            # persistent consts
            ropec = cp.tile([128, 4], f32, tag="ropec")
            sel2 = cp.tile([2, 128], f32, tag="sel2")
            ones128 = cp.tile([128, 1], f32, tag="ones128")
            ones1 = cp.tile([1, 128], f32, tag="ones1")
            ident4 = cp.tile([4, 4], bf16, tag="ident4")
            ident128 = cp.tile([128, 128], bf16, tag="ident128")
            for t_, n_ in [(ropec, 'ropec'), (sel2, 'sel2'), (ones128, 'ones128'),
                           (ones1, 'ones1'), (ident4, 'ident4'), (ident128, 'ident128')]:
                nc.sync.dma_start(out=t_[:], in_=din[n_][:, :])

            # junction landing buffers: slot 0 = own partial, slots 1..7 = peers
            # (XOR-relative: slot k holds the partial of a distinct peer).
            # Double buffered across junctions; the natural dataflow makes two
            # buffers race-free.
            jbufs = [cp.tile([128, 8, 16], f32, name=f"jbuf{v}", tag=f"jbuf{v}")
                     for v in range(2)]

            # zero-once buffers for attention
            qbd = cp.tile([128, 2, 2], bf16, tag="qbd")        # block-diag q per batch
            awT = cp.tile([128, T, 4], bf16, tag="awT")        # transposed exp-scores
            eps_t = cp.tile([1, 1], f32, tag="eps")
            nc.vector.memset(qbd[:], 0.0)
            nc.vector.memset(awT[:], 0.0)
            nc.vector.memset(eps_t[:], EPS)

            x = wk.tile([128, 8, 2], f32, tag="x")
            nc.sync.dma_start(out=x[:], in_=din['xT'][:, :, :])

            # warm up the collectives path: the first ncfw trigger pays a
            # ~100us one-time runtime-staging cost; absorb it under the
            # weight-DMA cold start instead of stalling junction 0
            wu_in = dp.tile([1, 4], f32, tag="wu_in")
            wu_out = dp.tile([8, 4], f32, tag="wu_out")
            nc.gpsimd.collective_compute(
                "AllGather", ALU.bypass,
                replica_groups=[list(range(NC))],
                ins=[wu_in.opt()], outs=[wu_out.opt()],
            )

            def rmsnorm_pre(x_t, small_t, w_off):
                """Deferred-rsq rmsnorm: returns (xw bf16 = x*n_w, rsq f32
                [128,2]). The GEMV may start on xw immediately; multiply its
                (tiny) output by rsq afterwards -- the sqrt chain overlaps."""
                xsq = wk.tile([128, 8, 2], f32, tag="xsq")
                nc.vector.tensor_mul(xsq[:], x_t[:], x_t[:])
                pss = pp.tile([1, 8, 2], f32, tag="misc")
                nc.tensor.matmul(pss[:], ones128[:], xsq[:].rearrange("p t b -> p (t b)"),
                                 start=True, stop=True)
                ss2 = wk.tile([1, 2], f32, tag="ss2")
                nc.vector.tensor_reduce(ss2[:], pss[:].rearrange("p t b -> p b t"),
                                        axis=AX.X, op=ALU.add)
                nc.scalar.activation(ss2[:], ss2[:], AF.Sqrt, bias=eps_t[:], scale=1.0 / H)
                psb = pp.tile([128, 2], f32, name="psb", tag="misc")
                nc.tensor.matmul(psb[:], ones1[:], ss2[:], start=True, stop=True)
                rsq = wk.tile([128, 2], f32, tag="rsq")
                nc.vector.reciprocal(rsq[:], psb[:])
                xw = wk.tile([128, 8, 2], bf16, tag="xn")
                for b in range(2):
                    nc.vector.tensor_tensor(xw[:, :, b], x_t[:, :, b],
                                            small_t[:, w_off:w_off + 8], op=ALU.mult)
                return xw, rsq

            JJ = [0]
            # waits attached AFTER TileContext exits: the tile scheduling sim
            # cannot model remote semaphore increments (would flag deadlock)
            pending_waits = []

            def junction_cc(psum_p, x_t, add_ob, small_t):
                """Fallback: AllGather partials via ncfw collective + local sum."""
                stage = wk.tile([128, 16], f32, tag="jstage")
                nc.vector.tensor_copy(stage[:], psum_p[:].rearrange("p t b -> p (t b)"))
                jin = dp.tile([128, 16], f32, tag="jin")
                # SWDGE (gpsimd) DMA: the collective trigger runs on gpsimd
                # too, so the done-sem wait is a same-engine poll instead of a
                # ~2us cross-engine wakeup before PSEUDO_TRIGGER_COLLECTIVE
                nc.gpsimd.dma_start(out=jin[:], in_=stage[:])
                jout = dp.tile([1024, 16], f32, tag="jout")
                nc.gpsimd.collective_compute(
                    "AllGather", ALU.bypass,
                    replica_groups=[list(range(NC))],
                    ins=[jin.opt()], outs=[jout.opt()],
                )
                land = wk.tile([128, 8, 16], f32, tag="land")
                nc.sync.dma_start(out=land[:], in_=jout[:, :].rearrange("(r p) f -> p r f", p=128))
                txo = None
                if add_ob:
                    txo = wk.tile([128, 16], f32, tag="txo")
                    nc.vector.tensor_add(txo[:], x_t[:].rearrange("p t b -> p (t b)"),
                                         small_t[:, SW_OB:SW_OB + 16])
                red = wk.tile([128, 16], f32, tag="red")
                nc.vector.tensor_reduce(red[:], land[:].rearrange("p r f -> p f r"),
                                        axis=AX.X, op=ALU.add)
                nx = wk.tile([128, 8, 2], f32, tag="x")
                nc.vector.tensor_add(nx[:].rearrange("p t b -> p (t b)"), red[:],
                                     txo[:] if txo is not None else
                                     x_t[:].rearrange("p t b -> p (t b)"))
                return nx

            def junction(psum_p, x_t, add_ob, small_t):
                """Cross-core sum of partials via SBUF->SBUF remote-DMA broadcast."""
                if not USE_RDMA:
                    return junction_cc(psum_p, x_t, add_ob, small_t)
                jj = JJ[0]
                JJ[0] += 1
                buf = jbufs[jj % 2]
                # Preps read buf[:,0] (in_ap) -> Tile RAW edge on this copy
                # gates the whole Pool stream (preps, then trigger).
                nc.vector.tensor_copy(buf[:, 0, :], psum_p[:].rearrange("p t b -> p (t b)"))
                for k in range(1, NC):
                    rd = [None] * 8
                    rd[k] = (0, k)
                    nc.gpsimd.remote_dma_broadcast(
                        out_ap=buf[:, k, :], in_ap=buf[:, 0, :],
                        remote_sem=rsem, local_sem=lsem, rdests=rd)
                nc.gpsimd.trigger_dma(count=None)
                txo = None
                if add_ob:
                    txo = wk.tile([128, 16], f32, tag="txo")
                    nc.vector.tensor_add(txo[:], x_t[:].rearrange("p t b -> p (t b)"),
                                         small_t[:, SW_OB:SW_OB + 16])
                red = wk.tile([128, 16], f32, tag="red")
                rinst = nc.vector.tensor_reduce(red[:], buf[:].rearrange("p s f -> p f s"),
                                                axis=AX.X, op=ALU.add)
                pending_waits.append((rinst, rsem, 14 * (jj + 1)))
                nx = wk.tile([128, 8, 2], f32, tag="x")
                nc.vector.tensor_add(nx[:].rearrange("p t b -> p (t b)"), red[:],
                                     txo[:] if txo is not None else
                                     x_t[:].rearrange("p t b -> p (t b)"))
                return nx

            att_idx = 0
            for i in range(N_LAYERS):
                bt = BLOCK[i]
                pwT = wp.tile([128, 8, 384], bf16, tag="pwT")
                nc.sync.dma_start(out=pwT[:], in_=din[f'pwT_{i}'][:, :, :])
                small = wp.tile([128, SMALL_W], f32, tag="small")
                nc.sync.dma_start(out=small[:], in_=din[f'small_{i}'][:, :])
                owT = wp.tile([128, 1024], bf16, tag="owT")
                nc.sync.dma_start(out=owT[:], in_=din[f'owT_{i}'][:, :])
                m1T = wp.tile([128, 8, 256], bf16, tag="m1T")
                nc.sync.dma_start(out=m1T[:], in_=din[f'm1T_{i}'][:, :, :])
                m2T = wp.tile([128, 2, 1024], bf16, tag="m2T")
                nc.sync.dma_start(out=m2T[:], in_=din[f'm2T_{i}'][:, :, :])
                if bt == 'ATT':
                    kT = ap_.tile([128, 2 * W], bf16, tag="kT")
                    nc.sync.dma_start(out=kT[:], in_=din[f'kT_{att_idx}'][:, :])
                    vv = ap_.tile([128, T, 2, 2, 64], bf16, tag="vv")
                    nc.sync.dma_start(out=vv[:], in_=din[f'v_{att_idx}'][:, :].rearrange(
                        "p (t b h d) -> p t b h d", t=T, b=2, h=2))

                xn, rsq1 = rmsnorm_pre(x, small, SW_N1)

                # proj: z_raw^T [128, 3(m), 2(b)] (norm scale applied below)
                pz_raw = pp.tile([128, 3, 2], f32, tag="zh")
                for m in range(3):
                    for kt in range(8):
                        nc.tensor.matmul(pz_raw[:, m, :], pwT[:, kt, m * 128:(m + 1) * 128],
                                         xn[:, kt, :], start=(kt == 0), stop=(kt == 7))
                pz = wk.tile([128, 3, 2], f32, tag="zs")
                for m in range(3):
                    nc.vector.tensor_mul(pz[:, m, :], pz_raw[:, m, :], rsq[:] if False else rsq1[:])

                if bt != 'ATT':
                    # fir1 on each of x2|x1|v tiles: zp = w2*u + s0*w0 + s1*w1 + sfb'
                    zp = wk.tile([128, 3, 2], f32, tag="zp")
                    tt = wk.tile([128, 2], f32, tag="tt")
                    for m in range(3):
                        nc.vector.tensor_scalar(tt[:], pz[:, m, :],
                                                small[:, SW_SFW + 3 * m + 2:SW_SFW + 3 * m + 3], small[:, SW_SFB + m:SW_SFB + m + 1],
                                                op0=ALU.mult, op1=ALU.add)
                        nc.vector.scalar_tensor_tensor(
                            tt[:], small[:, SW_FS + 4 * m:SW_FS + 4 * m + 4:2],
                            small[:, SW_SFW + 3 * m:SW_SFW + 3 * m + 1], tt[:], op0=ALU.mult, op1=ALU.add)
                        nc.vector.scalar_tensor_tensor(
                            zp[:, m, :], small[:, SW_FS + 4 * m + 1:SW_FS + 4 * m + 5:2],
                            small[:, SW_SFW + 3 * m + 1:SW_SFW + 3 * m + 2], tt[:], op0=ALU.mult, op1=ALU.add)
                    x1v = wk.tile([128, 2], f32, tag="x1v")
                    nc.vector.tensor_mul(x1v[:], zp[:, 1, :], zp[:, 2, :])

                    y2 = wk.tile([128, 2], bf16, tag="y2")
                    if bt == 'HCS':
                        acc = wk.tile([128, 2], f32, tag="acc")
                        yb = wk.tile([128, 2], f32, tag="yb")
                        scratch = wk.tile([128, 6], f32, tag="scr6")
                        nc.vector.tensor_scalar(yb[:], x1v[:], small[:, SW_S2 + 6:SW_S2 + 7],
                                                small[:, SW_S2 + 7:SW_S2 + 8],
                                                op0=ALU.mult, op1=ALU.add)
                        for b in range(2):
                            nc.vector.tensor_mul(scratch[:], small[:, SW_S2 + 8 + 6 * b:SW_S2 + 14 + 6 * b],
                                                 small[:, SW_S2:SW_S2 + 6])
                            nc.vector.tensor_reduce(acc[:, b:b + 1], scratch[:], axis=AX.X, op=ALU.add)
                        nc.vector.tensor_add(yb[:], yb[:], acc[:])
                        nc.vector.tensor_mul(y2[:], yb[:], zp[:, 0, :])
                    elif bt == 'HCM':
                        acc = wk.tile([128, 2], f32, tag="acc")
                        yb = wk.tile([128, 2], f32, tag="yb")
                        scratch = wk.tile([128, 127], f32, tag="scr127")
                        for b in range(2):
                            nc.vector.tensor_mul(scratch[:], small[:, SW_S2 + 128 + 127 * b:SW_S2 + 255 + 127 * b],
                                                 small[:, SW_S2:SW_S2 + 127])
                            nc.vector.tensor_reduce(acc[:, b:b + 1], scratch[:], axis=AX.X, op=ALU.add)
                        nc.vector.scalar_tensor_tensor(yb[:], x1v[:], small[:, SW_S2 + 127:SW_S2 + 128],
                                                       acc[:], op0=ALU.mult, op1=ALU.add)
                        nc.vector.tensor_mul(y2[:], yb[:], zp[:, 0, :])
                    else:  # HCL
                        dx = wk.tile([128, 2], f32, tag="dx")
                        nc.vector.tensor_scalar_mul(dx[:], x1v[:], small[:, SW_S2 + 32:SW_S2 + 33])
                        t1 = wk.tile([128, 16], f32, tag="t1")
                        iirn = wk.tile([128, 16], f32, tag="iirn")
                        res = wk.tile([128, 2], f32, tag="res")
                        for b in range(2):
                            nc.vector.tensor_mul(t1[:], small[:, SW_S2 + 33 + 16 * b:SW_S2 + 49 + 16 * b],
                                                 small[:, SW_S2:SW_S2 + 16])
                            nc.vector.tensor_scalar_add(iirn[:], t1[:], x1v[:, b:b + 1])
                            nc.vector.tensor_mul(t1[:], iirn[:], small[:, SW_S2 + 16:SW_S2 + 32])
                            nc.vector.tensor_reduce(res[:, b:b + 1], t1[:], axis=AX.X, op=ALU.add)
                        nc.vector.tensor_add(res[:], res[:], dx[:])
                        nc.vector.tensor_mul(y2[:], res[:], zp[:, 0, :])
                elif STUB_ATT:
                    y2 = wk.tile([128, 2], bf16, tag="y2")
                    nc.vector.tensor_copy(y2[:], pz[:, 0, :])
                else:
                    # ---- attention ----
                    q_sb = wk.tile([128, 2], f32, tag="q_sb")
                    k_sb = wk.tile([128, 2], f32, tag="k_sb")
                    v_sb = wk.tile([128, 2], bf16, tag="v_sb")
                    for m, dst in ((0, q_sb), (1, k_sb), (2, v_sb)):
                        nc.vector.tensor_scalar_add(dst[:], pz[:, m, :], small[:, SW_SFB + m:SW_SFB + m + 1])

                    def rope(src, c_col, s_col, dt, nm):
                        tmp = wk.tile([128, 2], f32, tag="rtmp")
                        for base in (0, 64):
                            nc.vector.tensor_copy(tmp[base:base + 32, :], src[base + 32:base + 64, :])
                            nc.vector.tensor_copy(tmp[base + 32:base + 64, :], src[base:base + 32, :])
                        nc.vector.tensor_scalar_mul(tmp[:], tmp[:], ropec[:, s_col:s_col + 1])
                        dst = wk.tile([128, 2], dt, name=nm, tag=nm)
                        nc.vector.scalar_tensor_tensor(dst[:], src[:], ropec[:, c_col:c_col + 1], tmp[:],
                                                       op0=ALU.mult, op1=ALU.add)
                        return dst

                    qr = rope(q_sb, 0, 1, f32, "qr")
                    kr = rope(k_sb, 2, 3, bf16, "kr")

                    nc.vector.tensor_copy(qbd[0:64, 0, 0:1], qr[0:64, 0:1])
                    nc.vector.tensor_copy(qbd[64:128, 0, 1:2], qr[64:128, 0:1])
                    nc.vector.tensor_copy(qbd[0:64, 1, 0:1], qr[0:64, 1:2])
                    nc.vector.tensor_copy(qbd[64:128, 1, 1:2], qr[64:128, 1:2])

                    pscs = [pp.tile([2, 1024], f32, name=f"psc{b}", tag=f"sc{b}") for b in range(2)]
                    for b in range(2):
                        for c0 in range(0, W, 512):
                            cw = min(512, W - c0)
                            nc.tensor.matmul(pscs[b][:, c0:c0 + cw],
                                             qbd[:, b, :], kT[:, b * W + c0:b * W + c0 + cw],
                                             start=True, stop=True)
                    pcur = pp.tile([2, 2], f32, name="pcur", tag="misc")
                    for b in range(2):
                        nc.tensor.matmul(pcur[:, b:b + 1], qbd[:, b, :], kr[:, b:b + 1],
                                         start=True, stop=True)

                    escs = [wk.tile([2, W + 1], bf16, name=f"esc{b}", tag=f"esc{b}") for b in range(2)]
                    rec = wk.tile([2, 2], f32, tag="rec")
                    mx = wk.tile([2, 2], f32, tag="mx")
                    nm = wk.tile([2, 2], f32, tag="nm")
                    se = wk.tile([2, 2], f32, tag="se")
                    ecur = wk.tile([2, 2], f32, tag="ecur")
                    for b in range(2):
                        nc.vector.tensor_reduce(mx[:, b:b + 1], pscs[b][:, 0:W], axis=AX.X, op=ALU.max)
                        nc.vector.tensor_tensor(mx[:, b:b + 1], mx[:, b:b + 1], pcur[:, b:b + 1], op=ALU.max)
                        nc.vector.tensor_scalar_mul(nm[:, b:b + 1], mx[:, b:b + 1], -1.0)
                        nc.scalar.activation(escs[b][:, 0:W], pscs[b][:, 0:W], AF.Exp,
                                             bias=nm[:, b:b + 1], scale=1.0, accum_out=se[:, b:b + 1])
                        nc.scalar.activation(escs[b][:, W:W + 1], pcur[:, b:b + 1], AF.Exp,
                                             bias=nm[:, b:b + 1], scale=1.0)
                        nc.vector.tensor_copy(ecur[:, b:b + 1], escs[b][:, W:W + 1])
                        nc.vector.tensor_add(se[:, b:b + 1], se[:, b:b + 1], ecur[:, b:b + 1])
                        nc.vector.reciprocal(rec[:, b:b + 1], se[:, b:b + 1])

                    pvt = pp.tile([1, 256], bf16, name="pvt", tag="misc")
                    for b in range(2):
                        nc.tensor.matmul(pvt[:, b * 128:(b + 1) * 128],
                                         v_sb[:, b:b + 1], ident128[:],
                                         is_transpose=True, start=True, stop=True)
                    nc.vector.tensor_copy(
                        vv[rem:rem + 1, T - 1, :, :, :].rearrange("p b h d -> p (b h d)"),
                        pvt[:, :])

                    for t_ in range(T):
                        c0 = t_ * 128
                        cwid = min(128, W + 1 - c0)
                        for b in range(2):
                            pat = pp.tile([128, 2], bf16, name=f"pat{b}", tag="misc")
                            nc.tensor.matmul(pat[0:cwid, :], escs[b][:, c0:c0 + cwid],
                                             ident4[0:2, 0:2],
                                             is_transpose=True, start=True, stop=True)
                            nc.vector.tensor_copy(awT[0:cwid, t_, 2 * b:2 * b + 2], pat[0:cwid, :])

                    py = pp.tile([128, 2], f32, name="py", tag="ctxy")
                    for b in range(2):
                        for h in range(2):
                            for t_ in range(T):
                                nc.tensor.matmul(py[h * 64:(h + 1) * 64, b:b + 1],
                                                 vv[:, t_, b, h, :], awT[:, t_, b * 2 + h:b * 2 + h + 1],
                                                 start=(t_ == 0), stop=(t_ == T - 1))

                    prb = pp.tile([128, 2], f32, name="prb", tag="misc")
                    nc.tensor.matmul(prb[:], sel2[:], rec[:], start=True, stop=True)
                    rb = wk.tile([128, 2], f32, tag="rbs")
                    nc.vector.tensor_copy(rb[:], prb[:])
                    y2 = wk.tile([128, 2], bf16, tag="y2")
                    nc.vector.tensor_mul(y2[:], py[:], rb[:])

                # out proj: partial^T [128, 8(t), 2(b)]
                pp1 = pp.tile([128, 8, 2], f32, tag="pj")
                for m in range(8):
                    nc.tensor.matmul(pp1[:, m, :], owT[:, m * 128:(m + 1) * 128], y2[:],
                                     start=True, stop=True)
                x_mid = junction(pp1, x, True, small)

                # mlp
                xn2, rsq2 = rmsnorm_pre(x_mid, small, SW_N2)
                ph = pp.tile([128, 2, 2], f32, name="ph", tag="zh")
                for m in range(2):
                    for kt in range(8):
                        nc.tensor.matmul(ph[:, m, :], m1T[:, kt, m * 128:(m + 1) * 128],
                                         xn2[:, kt, :], start=(kt == 0), stop=(kt == 7))
                hs = wk.tile([128, 2, 2], f32, tag="hs")
                for m in range(2):
                    nc.vector.tensor_mul(hs[:, m, :], ph[:, m, :], rsq2[:])
                hg = wk.tile([128, 2, 2], bf16, tag="hg")
                for m in range(2):
                    nc.scalar.activation(hg[:, m, :], hs[:, m, :], AF.Gelu_apprx_tanh)
                pp2 = pp.tile([128, 8, 2], f32, name="pp2", tag="pj")
                for m in range(8):
                    for kt in range(2):
                        nc.tensor.matmul(pp2[:, m, :], m2T[:, kt, m * 128:(m + 1) * 128],
                                         hg[:, kt, :], start=(kt == 0), stop=(kt == 1))
                x = junction(pp2, x_mid, False, small)

                if bt == 'ATT':
                    att_idx += 1

            for b in range(2):
                nc.sync.dma_start(out=out_t.ap()[b].rearrange("(t p) -> p t", p=128),
                                  in_=x[:, :, b])

    # attach the remote-arrival gates now that the tile scheduler has run;
    # check=False: extra waits are split onto EventSemaphores by compile()
    for rinst, sem, thresh in pending_waits:
        rinst.wait_op(sem, thresh, "sem-ge", check=False)

    nc.compile()
    return nc


_CACHE = {}


def kernel(**inputs):
    pos = int(np.asarray(inputs['position']))
    if pos not in _CACHE:
        _CACHE[pos] = _build(pos)
    nc = _CACHE[pos]
    in_maps = [_prep_core_inputs(inputs, c, pos) for c in range(NC)]
    res = bass_utils.run_bass_kernel_spmd(nc, in_maps, core_ids=list(range(NC)))
    out = res.results[0]['out']  # [2, 1024], replicated across cores
    return out.reshape(B, 1, H).astype(np.float32)
